# revision 26
# baseline (speedup 1.0000x reference)
"""Trainium2 Bass kernel for nn_Classical_autoencoder (patch MLP autoencoder + cosine fold).

Contract: kernel(**inputs) takes FULL inputs (img (32,1,512,512), W1 (16,4), b1 (4,),
W2 (4,4), b2 (4,), W3 (4,16), b3 (16,)) and returns the FULL (32,512,512) output.
Internally: pure data-parallel over 8 NeuronCores, 4 images per core.

v5 design:
  - host precomputes the full im2col tensor X [128, ci, l, t, j] (all matmul /
    elementwise reads contiguous) and inx = 1/(4*max(|x|,1e-8)) per patch in
    fold layout; inx rides in the same DRAM tensor (one DMA per image)
  - contraction weights are per-ci one-hot maps (g -> partition 4g+ci) so the
    accumulated dot / |y|^2 land directly in fold layout: one sim pipeline per
    image, no DRAM bounce / realign DMA
  - row-fold (T[u] = R[u-1]+R[u]) done with 4 small bf16 matmuls on the PE
    (shift/identity weights) instead of partition-shift DMAs
  - deep software pipeline: every PE instruction's producers are >=1 macro-iter
    old; single 5-deep psum ring for z1/z2/z3 frees a bank for the fold
  - const DMAs issued from the scalar HWDGE queue in parallel with X DMAs
"""

import sys

for _p in ("/opt/trn_rl_repo", "/root/.axon_site/_ro/trn_rl_repo"):
    if _p not in sys.path:
        sys.path.append(_p)

from contextlib import ExitStack

import numpy as np

import concourse.bass as bass
import concourse.tile as tile
from concourse import bacc, mybir

F32 = mybir.dt.float32
BF16 = mybir.dt.bfloat16
ALU = mybir.AluOpType
ACT = mybir.ActivationFunctionType

IMG = 512
NJ = 256  # padded patch-column count
XEL = 4 * 4 * 2 * NJ  # X elements per partition (8192)
NSAMP = 4
NCORES = 8
NIT = NSAMP * 4


def build_nc() -> bass.Bass:
    nc = bacc.Bacc()

    # [s, p, ci, l, t, j] im2col (8192) then inx (512)
    x_d = nc.declare_dram_parameter("xim", [NSAMP, 128, XEL + 512], BF16, isOutput=False)[:]
    # bf16 consts: l1w(512) l2w(128) | l3w(512) cwv(512) foldw(512) = 2176
    cb_d = nc.declare_dram_parameter("cbf", [128, 2176], BF16, isOutput=False)[:]
    cf_d = nc.declare_dram_parameter("cf32", [128, 6], F32, isOutput=False)[:]
    out4 = nc.declare_dram_parameter("out4", [NSAMP, IMG, IMG], F32, isOutput=True)[:]

    with ExitStack() as ctx:
        tc = ctx.enter_context(tile.TileContext(nc))
        consts = ctx.enter_context(tc.tile_pool(name="consts", bufs=1))
        rows = ctx.enter_context(tc.tile_pool(name="rows", bufs=3))
        mlp = ctx.enter_context(tc.tile_pool(name="mlp", bufs=3))
        fold = ctx.enter_context(tc.tile_pool(name="fold", bufs=2))
        ps = ctx.enter_context(tc.tile_pool(name="ps", bufs=1, space="PSUM"))

        xt = {}
        inxt = {}

        def dma_img(s, first=False):
            if s >= NSAMP:
                return
            xx = rows.tile([128, XEL + 512], BF16, tag="xx", name="xx")
            if first:  # split so the first matmuls only wait for ci=0's chunk
                nc.sync.dma_start(out=xx[:, 0 : 8 * NJ], in_=x_d[s, :, 0 : 8 * NJ])
                nc.sync.dma_start(out=xx[:, 8 * NJ :], in_=x_d[s, :, 8 * NJ :])
            else:
                nc.sync.dma_start(out=xx, in_=x_d[s, :, :])
            xt[s] = xx[:, 0:XEL].rearrange("p (ci l t j) -> p ci l t j", ci=4, l=4, t=2)
            inxt[s] = xx[:, XEL:].rearrange("p (t j) -> p t j", t=2)

        cb = consts.tile([128, 2176], BF16)
        nc.scalar.dma_start(out=cb[:, 0:640], in_=cb_d[:, 0:640])  # l1w+l2w first
        dma_img(0, first=True)
        cf = consts.tile([128, 6], F32)
        nc.scalar.dma_start(out=cf, in_=cf_d[:, :])
        nc.scalar.dma_start(out=cb[:, 640:], in_=cb_d[:, 640:])
        dma_img(1)  # issued last: image 0's chunks win the HBM bandwidth race
        cbr = cb.rearrange("p (a b) -> p a b", a=17)
        l1w = cbr[:, 0:4]
        l2w = cbr[:, 4]
        l3w = cbr[:, 5:9]
        cwv = cbr[:, 9:13]
        fwD = cbr[:, 13]  # diag(2,1,..,1)
        fwC = cbr[:, 14]  # [p,m]=1 iff p==m-1
        fwA = cbr[:, 15]  # diag(1,..,1,2)
        fwB = cbr[:, 16]  # diag(1,..,1,0)
        b3v = cf[:, 0:4]
        b1v = cf[:, 4:5]
        b2v = cf[:, 5:6]
        epsv = consts.tile([128, 1], F32)
        nc.vector.memset(epsv, 1e-12)

        h1t = {}
        h2t = {}
        pt = {}
        ctdt = {}
        ctyt = {}
        p1t = {}

        def stageA(k):  # z1 + h1
            s, ci = divmod(k, 4)
            z1 = ps.tile([128, 2, NJ], F32, tag="zz", bufs=5, name="z1")
            for l in range(4):
                nc.tensor.matmul(z1, l1w[:, l, :], xt[s][:, ci, l], start=(l == 0), stop=(l == 3))
            h1 = mlp.tile([128, 2, NJ], BF16, tag="h1")
            nc.scalar.activation(h1, z1, ACT.Relu, bias=b1v)
            h1t[k] = h1

        def stageB(k):  # z2 + h2
            z2 = ps.tile([128, 2, NJ], F32, tag="zz", bufs=5, name="z2")
            nc.tensor.matmul(z2, l2w, h1t.pop(k), start=True, stop=True)
            h2 = mlp.tile([128, 2, NJ], BF16, tag="h2")
            nc.scalar.activation(h2, z2, ACT.Relu, bias=b2v)
            h2t[k] = h2

        def stageC(k):  # z3, yv, prod, ysq
            s, ci = divmod(k, 4)
            h2 = h2t.pop(k)
            z3s = []
            for l in range(4):
                z3 = ps.tile([128, 2, NJ], F32, tag="zz", bufs=5, name="z3")
                nc.tensor.matmul(z3, l3w[:, l, :], h2, start=True, stop=True)
                z3s.append(z3)
            Y = mlp.tile([128, 4, 2, NJ], BF16, tag="Y")
            nc.scalar.activation(Y[:, 0], z3s[0], ACT.Relu, bias=b3v[:, 0:1])
            nc.scalar.activation(Y[:, 1], z3s[1], ACT.Relu, bias=b3v[:, 1:2])
            nc.scalar.activation(Y[:, 2], z3s[2], ACT.Relu, bias=b3v[:, 2:3])
            nc.vector.tensor_scalar(Y[:, 3], z3s[3], b3v[:, 3:4], 0.0, ALU.add, ALU.max)
            P = mlp.tile([128, 4, 2, NJ], BF16, tag="P")
            nc.vector.tensor_tensor(P, xt[s][:, ci], Y, ALU.mult)
            YS = mlp.tile([128, 4, 2, NJ], BF16, tag="YS")
            nc.vector.tensor_tensor(YS, Y, Y, ALU.mult)
            pt[k] = (P, YS)

        def stageD(k):  # contractions (issued one iter late; producers all ready)
            s, ci = divmod(k, 4)
            P, YS = pt.pop(k)
            if ci == 0:
                ctdt[s] = ps.tile([128, 2, NJ], F32, tag="ctd", bufs=1, name="ctd")
                ctyt[s] = ps.tile([128, 2, NJ], F32, tag="cty", bufs=1, name="cty")
            ctd, cty = ctdt[s], ctyt[s]
            for l in range(4):
                nc.tensor.matmul(
                    ctd, cwv[:, ci, :], P[:, l],
                    start=(ci == 0 and l == 0), stop=(ci == 3 and l == 3),
                )
            for l in range(4):
                nc.tensor.matmul(
                    cty, cwv[:, ci, :], YS[:, l],
                    start=(ci == 0 and l == 0), stop=(ci == 3 and l == 3),
                )

        def tail1(s):  # prefetch, sim pipeline, col fold
            dma_img(s + 2)
            ctd = ctdt.pop(s)
            cty = ctyt.pop(s)
            u = fold.tile([128, 2, NJ], F32, tag="u")
            nc.vector.tensor_tensor(u, ctd, inxt[s], ALU.mult)
            av = fold.tile([128, 2, NJ], F32, tag="av")
            nc.vector.tensor_scalar(av, cty, epsv[:, :], None, ALU.add)
            q = fold.tile([128, 2, NJ], F32, tag="q")
            nc.vector.reciprocal_approx_fast(q, av)
            sq = fold.tile([128, 2, NJ], F32, tag="sq")
            nc.scalar.activation(sq, q, ACT.Sqrt)  # 1/|y|
            eng = nc.vector if s == NSAMP - 1 else nc.gpsimd  # fast path on tail
            simt = fold.tile([128, 2, NJ], BF16, tag="simt")
            eng.tensor_tensor(simt, u, sq, ALU.mult)

            rf = fold.tile([128, 2, NJ], BF16, tag="rf")
            eng.tensor_tensor(
                rf[:, :, 1:255], simt[:, :, 0:254], simt[:, :, 1:255], ALU.add
            )
            nc.scalar.activation(rf[:, :, 0:1], simt[:, :, 0:1], ACT.Copy, scale=2.0)
            nc.scalar.activation(
                rf[:, :, 255:256], simt[:, :, 254:255], ACT.Copy, scale=2.0
            )
            p1t[s] = rf

        def tail2(s):  # row fold on PE + col-duplicate + store
            rf = p1t.pop(s)
            tf = ps.tile([128, 2, NJ], F32, tag="tfp", bufs=1, name="tf")
            # tf[:,0][m] = rf[m-1,1] + rf[m,0] (m=0: 2*rf[0,0])
            nc.tensor.matmul(tf[:, 0], fwD, rf[:, 0, :], start=True, stop=False)
            nc.tensor.matmul(tf[:, 0], fwC, rf[:, 1, :], start=False, stop=True)
            # tf[:,1][m] = rf[m,0] + rf[m,1] (m=127: 2*rf[127,0])
            nc.tensor.matmul(tf[:, 1], fwA, rf[:, 0, :], start=True, stop=False)
            nc.tensor.matmul(tf[:, 1], fwB, rf[:, 1, :], start=False, stop=True)
            up2 = fold.tile([128, 2, 512], F32, tag="up2")
            up2r = up2.rearrange("p lu (v cv) -> p lu cv v", cv=2)
            nc.vector.tensor_copy(up2r[:, :, 0, :], tf)
            nc.scalar.activation(up2r[:, :, 1, :], tf, ACT.Copy)
            # out rows r = 4p + 2lu + ru: row-duplicate = 2 DMAs reading up2
            for ru in range(2):
                nc.sync.dma_start(
                    out=bass.AP(
                        tensor=out4.tensor,
                        offset=out4.offset + s * IMG * IMG + ru * IMG,
                        ap=[[4 * IMG, 128], [2 * IMG, 2], [1, IMG]],
                    ),
                    in_=bass.AP(
                        tensor=up2.tensor,
                        offset=up2.offset,
                        ap=[[1024, 128], [512, 2], [1, 512]],
                    ),
                )

        # ---- deep software pipeline ----
        stageA(0)
        stageA(1)
        stageB(0)
        for k in range(NIT):
            stageC(k)
            if k + 1 < NIT:
                stageB(k + 1)
            if k + 2 < NIT:
                stageA(k + 2)
            if k - 1 >= 0:
                stageD(k - 1)
                s1, ci1 = divmod(k - 1, 4)
                if ci1 == 3:
                    tail1(s1)
            if k % 4 == 2 and k >= 6:
                tail2(k // 4 - 1)
        stageD(NIT - 1)
        tail1(NSAMP - 1)
        tail2(NSAMP - 1)

    nc.finalize()
    return nc


def make_weight_inputs(W1, b1, W2, b2, W3, b3):
    W1 = np.asarray(W1, np.float32)
    W2 = np.asarray(W2, np.float32)
    W3 = np.asarray(W3, np.float32)
    b1 = np.asarray(b1, np.float32)
    b2 = np.asarray(b2, np.float32)
    b3 = np.asarray(b3, np.float32)
    l1w = np.zeros((128, 4, 128), np.float32)
    l2w = np.zeros((128, 128), np.float32)
    l3w = np.zeros((128, 4, 128), np.float32)
    b3v = np.zeros((128, 4), np.float32)
    cwv = np.zeros((128, 4, 128), np.float32)
    for g in range(32):
        for k in range(4):
            for l in range(4):
                for c in range(4):
                    l1w[32 * k + g, l, 32 * c + g] = W1[4 * k + l, c]
                    l3w[32 * c + g, l, 32 * k + g] = W3[c, 4 * k + l]
                b3v[32 * k + g, l] = b3[4 * k + l]
            for ci in range(4):
                cwv[32 * k + g, ci, 4 * g + ci] = 1.0
        for c in range(4):
            for c2 in range(4):
                l2w[32 * c + g, 32 * c2 + g] = W2[c, c2]
    b1v = np.repeat(b1, 32).reshape(128, 1).astype(np.float32)
    b2v = np.repeat(b2, 32).reshape(128, 1).astype(np.float32)
    # fold-row weights
    fwD = np.diag(np.r_[2.0, np.ones(127)]).astype(np.float32)
    fwC = np.zeros((128, 128), np.float32)
    fwC[np.arange(127), np.arange(1, 128)] = 1.0  # [p, m]=1 iff p==m-1
    fwA = np.diag(np.r_[np.ones(127), 2.0]).astype(np.float32)
    fwB = np.diag(np.r_[np.ones(127), 0.0]).astype(np.float32)
    bf = _bf16()
    cbf = np.concatenate(
        [
            l1w.reshape(128, 512), l2w, l3w.reshape(128, 512), cwv.reshape(128, 512),
            fwD, fwC, fwA, fwB,
        ],
        axis=1,
    ).astype(bf)
    cf32 = np.concatenate([b3v, b1v, b2v], axis=1).astype(np.float32)
    return {"cbf": cbf, "cf32": cf32}


_NC = None


def get_nc():
    global _NC
    if _NC is None:
        _NC = build_nc()
    return _NC


def _bf16():
    import ml_dtypes

    return ml_dtypes.bfloat16


def gather_inputs(img_n):
    """(n,512,512) f32 -> xim (n,128,8704) bf16: im2col X then inx."""
    n = img_n.shape[0]
    pad = np.zeros((n, IMG + 4, IMG + 2), np.float32)
    pad[:, :IMG, :IMG] = img_n
    p = np.arange(128)
    li = np.arange(8)
    rows_idx = 16 * (p[:, None] % 32) + (p[:, None] // 32) + 2 * li[None, :]
    rws = pad[:, rows_idx, :]  # (n,128,8,514); li = 2ci+t
    rwr = rws.reshape(n, 128, 4, 2, IMG + 2)  # (ci, t, c)
    cols = 2 * np.arange(NJ)[None, :] + np.arange(4)[:, None]  # (l, j) -> 2j+l
    X = rwr[:, :, :, :, cols]  # (n,128,ci,t,l,j)
    X = X.transpose(0, 1, 2, 4, 3, 5)  # (n,128,ci,l,t,j)
    X = np.ascontiguousarray(X).reshape(n, 128, XEL)

    sq = img_n.astype(np.float64) ** 2
    p2 = sq[:, :, 0::2] + sq[:, :, 1::2]
    s4 = p2[:, :, 0:255] + p2[:, :, 1:256]
    r2 = s4[:, 0::2, :] + s4[:, 1::2, :]
    r4 = r2[:, 0:255, :] + r2[:, 1:256, :]  # (n,255,255) = |x|^2
    inx = np.zeros((n, 256, 256), np.float64)
    inx[:, :255, :255] = 1.0 / (4.0 * np.maximum(np.sqrt(r4), 1e-8))
    inx = inx.reshape(n, 128, 512)
    xim = np.concatenate([X, inx], axis=2).astype(_bf16())
    return xim


def build_in_maps(img, W1, b1, W2, b2, W3, b3):
    img = np.asarray(img, np.float32).reshape(32, IMG, IMG)
    wts = make_weight_inputs(W1, b1, W2, b2, W3, b3)
    in_maps = []
    for c in range(NCORES):
        m = {"xim": gather_inputs(img[c * NSAMP : (c + 1) * NSAMP])}
        m.update(wts)
        in_maps.append(m)
    return in_maps


def kernel(img, W1, b1, W2, b2, W3, b3):
    from concourse.bass_utils import run_bass_kernel_spmd

    nc = get_nc()
    in_maps = build_in_maps(img, W1, b1, W2, b2, W3, b3)
    res = run_bass_kernel_spmd(nc, in_maps, list(range(NCORES)))
    out = np.concatenate([res.results[i]["out4"] for i in range(NCORES)], axis=0)
    return out.astype(np.float32)


# revision 27
# speedup vs baseline: 1.1273x; 1.1273x over previous
"""Trainium2 Bass kernel for nn_Classical_autoencoder (patch MLP autoencoder + cosine fold).

Contract: kernel(**inputs) takes FULL inputs (img (32,1,512,512), W1 (16,4), b1 (4,),
W2 (4,4), b2 (4,), W3 (4,16), b3 (16,)) and returns the FULL (32,512,512) output.
Internally: pure data-parallel over 8 NeuronCores, 4 images per core.

v5 design:
  - host precomputes the full im2col tensor X [128, ci, l, t, j] (all matmul /
    elementwise reads contiguous) and inx = 1/(4*max(|x|,1e-8)) per patch in
    fold layout; inx rides in the same DRAM tensor (one DMA per image)
  - contraction weights are per-ci one-hot maps (g -> partition 4g+ci) so the
    accumulated dot / |y|^2 land directly in fold layout: one sim pipeline per
    image, no DRAM bounce / realign DMA
  - row-fold (T[u] = R[u-1]+R[u]) done with 4 small bf16 matmuls on the PE
    (shift/identity weights) instead of partition-shift DMAs
  - deep software pipeline: every PE instruction's producers are >=1 macro-iter
    old; single 5-deep psum ring for z1/z2/z3 frees a bank for the fold
  - const DMAs issued from the scalar HWDGE queue in parallel with X DMAs
"""

import sys

for _p in ("/opt/trn_rl_repo", "/root/.axon_site/_ro/trn_rl_repo"):
    if _p not in sys.path:
        sys.path.append(_p)

from contextlib import ExitStack

import numpy as np

import concourse.bass as bass
import concourse.tile as tile
from concourse import bacc, mybir

F32 = mybir.dt.float32
BF16 = mybir.dt.bfloat16
ALU = mybir.AluOpType
ACT = mybir.ActivationFunctionType

IMG = 512
NJ = 256  # padded patch-column count
XEL = 4 * 4 * 2 * NJ  # X elements per partition (8192)
NSAMP = 4
NCORES = 8
NIT = NSAMP * 4


def build_nc() -> bass.Bass:
    nc = bacc.Bacc()

    # [s, p, ci, l, t, j] im2col (8192) then inx (512)
    x_d = nc.declare_dram_parameter("xim", [NSAMP, 128, XEL + 512], BF16, isOutput=False)[:]
    # bf16 consts: l1w(512) l2w(128) | l3w(512) cwv(512) foldw(512) = 2176
    cb_d = nc.declare_dram_parameter("cbf", [128, 2176], BF16, isOutput=False)[:]
    cf_d = nc.declare_dram_parameter("cf32", [128, 6], F32, isOutput=False)[:]
    out4 = nc.declare_dram_parameter("out4", [NSAMP, IMG, IMG], F32, isOutput=True)[:]

    with ExitStack() as ctx:
        tc = ctx.enter_context(tile.TileContext(nc))
        consts = ctx.enter_context(tc.tile_pool(name="consts", bufs=1))
        rows = ctx.enter_context(tc.tile_pool(name="rows", bufs=3))
        mlp = ctx.enter_context(tc.tile_pool(name="mlp", bufs=3))
        fold = ctx.enter_context(tc.tile_pool(name="fold", bufs=2))
        ps = ctx.enter_context(tc.tile_pool(name="ps", bufs=1, space="PSUM"))

        xt = {}
        inxt = {}

        def dma_img(s, first=False):
            if s >= NSAMP:
                return
            xx = rows.tile([128, XEL + 512], BF16, tag="xx", name="xx")
            if first:  # split so the first matmuls only wait for ci=0's chunk
                nc.sync.dma_start(out=xx[:, 0 : 8 * NJ], in_=x_d[s, :, 0 : 8 * NJ])
                nc.sync.dma_start(out=xx[:, 8 * NJ :], in_=x_d[s, :, 8 * NJ :])
            else:
                nc.sync.dma_start(out=xx, in_=x_d[s, :, :])
            xt[s] = xx[:, 0:XEL].rearrange("p (ci l t j) -> p ci l t j", ci=4, l=4, t=2)
            inxt[s] = xx[:, XEL:].rearrange("p (t j) -> p t j", t=2)

        cb = consts.tile([128, 2176], BF16)
        nc.scalar.dma_start(out=cb[:, 0:640], in_=cb_d[:, 0:640])  # l1w+l2w first
        dma_img(0, first=True)
        cf = consts.tile([128, 6], F32)
        nc.scalar.dma_start(out=cf, in_=cf_d[:, :])
        nc.scalar.dma_start(out=cb[:, 640:], in_=cb_d[:, 640:])
        dma_img(1)  # issued last: image 0's chunks win the HBM bandwidth race
        cbr = cb.rearrange("p (a b) -> p a b", a=17)
        l1w = cbr[:, 0:4]
        l2w = cbr[:, 4]
        l3w = cbr[:, 5:9]
        cwv = cbr[:, 9:13]
        fwD = cbr[:, 13]  # diag(2,1,..,1)
        fwC = cbr[:, 14]  # [p,m]=1 iff p==m-1
        fwA = cbr[:, 15]  # diag(1,..,1,2)
        fwB = cbr[:, 16]  # diag(1,..,1,0)
        b3v = cf[:, 0:4]
        b1v = cf[:, 4:5]
        b2v = cf[:, 5:6]
        epsv = consts.tile([128, 1], F32)
        nc.vector.memset(epsv, 1e-12)

        h1t = {}
        h2t = {}
        pt = {}
        ctdt = {}
        ctyt = {}
        p1t = {}

        def stageA(k):  # z1 + h1
            s, ci = divmod(k, 4)
            z1 = ps.tile([128, 2, NJ], F32, tag="zz", bufs=5, name="z1")
            for l in range(4):
                nc.tensor.matmul(z1, l1w[:, l, :], xt[s][:, ci, l], start=(l == 0), stop=(l == 3))
            h1 = mlp.tile([128, 2, NJ], BF16, tag="h1")
            nc.scalar.activation(h1, z1, ACT.Relu, bias=b1v)
            h1t[k] = h1

        def stageB(k):  # z2 + h2
            z2 = ps.tile([128, 2, NJ], F32, tag="zz", bufs=5, name="z2")
            nc.tensor.matmul(z2, l2w, h1t.pop(k), start=True, stop=True)
            h2 = mlp.tile([128, 2, NJ], BF16, tag="h2")
            nc.scalar.activation(h2, z2, ACT.Relu, bias=b2v)
            h2t[k] = h2

        def stageC(k):  # z3, yv, prod, ysq
            s, ci = divmod(k, 4)
            h2 = h2t.pop(k)
            z3s = []
            for l in range(4):
                z3 = ps.tile([128, 2, NJ], F32, tag="zz", bufs=5, name="z3")
                nc.tensor.matmul(z3, l3w[:, l, :], h2, start=True, stop=True)
                z3s.append(z3)
            Y = mlp.tile([128, 4, 2, NJ], BF16, tag="Y")
            nc.scalar.activation(Y[:, 0], z3s[0], ACT.Relu, bias=b3v[:, 0:1])
            nc.scalar.activation(Y[:, 1], z3s[1], ACT.Relu, bias=b3v[:, 1:2])
            nc.scalar.activation(Y[:, 2], z3s[2], ACT.Relu, bias=b3v[:, 2:3])
            nc.vector.tensor_scalar(Y[:, 3], z3s[3], b3v[:, 3:4], 0.0, ALU.add, ALU.max)
            P = mlp.tile([128, 4, 2, NJ], BF16, tag="P")
            nc.vector.tensor_tensor(P, xt[s][:, ci], Y, ALU.mult)
            YS = mlp.tile([128, 4, 2, NJ], BF16, tag="YS")
            nc.vector.tensor_tensor(YS, Y, Y, ALU.mult)
            pt[k] = (P, YS)

        def stageD(k):  # contractions (issued one iter late; producers all ready)
            s, ci = divmod(k, 4)
            P, YS = pt.pop(k)
            if ci == 0:
                ctdt[s] = ps.tile([128, 2, NJ], F32, tag="ctd", bufs=1, name="ctd")
                ctyt[s] = ps.tile([128, 2, NJ], F32, tag="cty", bufs=1, name="cty")
            ctd, cty = ctdt[s], ctyt[s]
            for l in range(4):
                nc.tensor.matmul(
                    ctd, cwv[:, ci, :], P[:, l],
                    start=(ci == 0 and l == 0), stop=(ci == 3 and l == 3),
                )
            for l in range(4):
                nc.tensor.matmul(
                    cty, cwv[:, ci, :], YS[:, l],
                    start=(ci == 0 and l == 0), stop=(ci == 3 and l == 3),
                )

        def tail1(s):  # prefetch, sim pipeline, col fold
            dma_img(s + 2)
            ctd = ctdt.pop(s)
            cty = ctyt.pop(s)
            u = fold.tile([128, 2, NJ], F32, tag="u")
            nc.vector.tensor_tensor(u, ctd, inxt[s], ALU.mult)
            av = fold.tile([128, 2, NJ], F32, tag="av")
            nc.scalar.activation(av, cty, ACT.Identity, bias=epsv[:, :])
            q = fold.tile([128, 2, NJ], F32, tag="q")
            nc.vector.reciprocal_approx_fast(q, av)
            sq = fold.tile([128, 2, NJ], F32, tag="sq")
            nc.scalar.activation(sq, q, ACT.Sqrt)  # 1/|y|
            eng = nc.vector if s == NSAMP - 1 else nc.gpsimd  # fast path on tail
            simt = fold.tile([128, 2, NJ], BF16, tag="simt")
            eng.tensor_tensor(simt, u, sq, ALU.mult)

            rf = fold.tile([128, 2, NJ], BF16, tag="rf")
            eng.tensor_tensor(
                rf[:, :, 1:255], simt[:, :, 0:254], simt[:, :, 1:255], ALU.add
            )
            nc.scalar.activation(rf[:, :, 0:1], simt[:, :, 0:1], ACT.Copy, scale=2.0)
            nc.scalar.activation(
                rf[:, :, 255:256], simt[:, :, 254:255], ACT.Copy, scale=2.0
            )
            p1t[s] = rf

        def tail2(s):  # row fold on PE + col-duplicate + store
            rf = p1t.pop(s)
            tf = ps.tile([128, 2, NJ], F32, tag="tfp", bufs=1, name="tf")
            # tf[:,0][m] = rf[m-1,1] + rf[m,0] (m=0: 2*rf[0,0])
            nc.tensor.matmul(tf[:, 0], fwD, rf[:, 0, :], start=True, stop=False)
            nc.tensor.matmul(tf[:, 0], fwC, rf[:, 1, :], start=False, stop=True)
            # tf[:,1][m] = rf[m,0] + rf[m,1] (m=127: 2*rf[127,0])
            nc.tensor.matmul(tf[:, 1], fwA, rf[:, 0, :], start=True, stop=False)
            nc.tensor.matmul(tf[:, 1], fwB, rf[:, 1, :], start=False, stop=True)
            up2 = fold.tile([128, 2, 512], F32, tag="up2")
            up2r = up2.rearrange("p lu (v cv) -> p lu cv v", cv=2)
            nc.vector.tensor_copy(up2r[:, :, 0, :], tf)
            nc.scalar.activation(up2r[:, :, 1, :], tf, ACT.Copy)
            # out rows r = 4p + 2lu + ru: row-duplicate = 2 DMAs reading up2
            for ru in range(2):
                nc.sync.dma_start(
                    out=bass.AP(
                        tensor=out4.tensor,
                        offset=out4.offset + s * IMG * IMG + ru * IMG,
                        ap=[[4 * IMG, 128], [2 * IMG, 2], [1, IMG]],
                    ),
                    in_=bass.AP(
                        tensor=up2.tensor,
                        offset=up2.offset,
                        ap=[[1024, 128], [512, 2], [1, 512]],
                    ),
                )

        # ---- deep software pipeline ----
        stageA(0)
        stageA(1)
        stageB(0)
        for k in range(NIT):
            stageC(k)
            if k + 1 < NIT:
                stageB(k + 1)
            if k + 2 < NIT:
                stageA(k + 2)
            if k - 1 >= 0:
                stageD(k - 1)
                s1, ci1 = divmod(k - 1, 4)
                if ci1 == 3:
                    tail1(s1)
            if k % 4 == 2 and k >= 6:
                tail2(k // 4 - 1)
        stageD(NIT - 1)
        tail1(NSAMP - 1)
        tail2(NSAMP - 1)

    nc.finalize()
    return nc


def make_weight_inputs(W1, b1, W2, b2, W3, b3):
    W1 = np.asarray(W1, np.float32)
    W2 = np.asarray(W2, np.float32)
    W3 = np.asarray(W3, np.float32)
    b1 = np.asarray(b1, np.float32)
    b2 = np.asarray(b2, np.float32)
    b3 = np.asarray(b3, np.float32)
    l1w = np.zeros((128, 4, 128), np.float32)
    l2w = np.zeros((128, 128), np.float32)
    l3w = np.zeros((128, 4, 128), np.float32)
    b3v = np.zeros((128, 4), np.float32)
    cwv = np.zeros((128, 4, 128), np.float32)
    for g in range(32):
        for k in range(4):
            for l in range(4):
                for c in range(4):
                    l1w[32 * k + g, l, 32 * c + g] = W1[4 * k + l, c]
                    l3w[32 * c + g, l, 32 * k + g] = W3[c, 4 * k + l]
                b3v[32 * k + g, l] = b3[4 * k + l]
            for ci in range(4):
                cwv[32 * k + g, ci, 4 * g + ci] = 1.0
        for c in range(4):
            for c2 in range(4):
                l2w[32 * c + g, 32 * c2 + g] = W2[c, c2]
    b1v = np.repeat(b1, 32).reshape(128, 1).astype(np.float32)
    b2v = np.repeat(b2, 32).reshape(128, 1).astype(np.float32)
    # fold-row weights
    fwD = np.diag(np.r_[2.0, np.ones(127)]).astype(np.float32)
    fwC = np.zeros((128, 128), np.float32)
    fwC[np.arange(127), np.arange(1, 128)] = 1.0  # [p, m]=1 iff p==m-1
    fwA = np.diag(np.r_[np.ones(127), 2.0]).astype(np.float32)
    fwB = np.diag(np.r_[np.ones(127), 0.0]).astype(np.float32)
    bf = _bf16()
    cbf = np.concatenate(
        [
            l1w.reshape(128, 512), l2w, l3w.reshape(128, 512), cwv.reshape(128, 512),
            fwD, fwC, fwA, fwB,
        ],
        axis=1,
    ).astype(bf)
    cf32 = np.concatenate([b3v, b1v, b2v], axis=1).astype(np.float32)
    return {"cbf": cbf, "cf32": cf32}


_NC = None


def get_nc():
    global _NC
    if _NC is None:
        _NC = build_nc()
    return _NC


def _bf16():
    import ml_dtypes

    return ml_dtypes.bfloat16


def gather_inputs(img_n):
    """(n,512,512) f32 -> xim (n,128,8704) bf16: im2col X then inx."""
    n = img_n.shape[0]
    pad = np.zeros((n, IMG + 4, IMG + 2), np.float32)
    pad[:, :IMG, :IMG] = img_n
    p = np.arange(128)
    li = np.arange(8)
    rows_idx = 16 * (p[:, None] % 32) + (p[:, None] // 32) + 2 * li[None, :]
    rws = pad[:, rows_idx, :]  # (n,128,8,514); li = 2ci+t
    rwr = rws.reshape(n, 128, 4, 2, IMG + 2)  # (ci, t, c)
    cols = 2 * np.arange(NJ)[None, :] + np.arange(4)[:, None]  # (l, j) -> 2j+l
    X = rwr[:, :, :, :, cols]  # (n,128,ci,t,l,j)
    X = X.transpose(0, 1, 2, 4, 3, 5)  # (n,128,ci,l,t,j)
    X = np.ascontiguousarray(X).reshape(n, 128, XEL)

    sq = img_n.astype(np.float64) ** 2
    p2 = sq[:, :, 0::2] + sq[:, :, 1::2]
    s4 = p2[:, :, 0:255] + p2[:, :, 1:256]
    r2 = s4[:, 0::2, :] + s4[:, 1::2, :]
    r4 = r2[:, 0:255, :] + r2[:, 1:256, :]  # (n,255,255) = |x|^2
    inx = np.zeros((n, 256, 256), np.float64)
    inx[:, :255, :255] = 1.0 / (4.0 * np.maximum(np.sqrt(r4), 1e-8))
    inx = inx.reshape(n, 128, 512)
    xim = np.concatenate([X, inx], axis=2).astype(_bf16())
    return xim


def build_in_maps(img, W1, b1, W2, b2, W3, b3):
    img = np.asarray(img, np.float32).reshape(32, IMG, IMG)
    wts = make_weight_inputs(W1, b1, W2, b2, W3, b3)
    in_maps = []
    for c in range(NCORES):
        m = {"xim": gather_inputs(img[c * NSAMP : (c + 1) * NSAMP])}
        m.update(wts)
        in_maps.append(m)
    return in_maps


def kernel(img, W1, b1, W2, b2, W3, b3):
    from concourse.bass_utils import run_bass_kernel_spmd

    nc = get_nc()
    in_maps = build_in_maps(img, W1, b1, W2, b2, W3, b3)
    res = run_bass_kernel_spmd(nc, in_maps, list(range(NCORES)))
    out = np.concatenate([res.results[i]["out4"] for i in range(NCORES)], axis=0)
    return out.astype(np.float32)


# revision 28
# speedup vs baseline: 1.1648x; 1.0332x over previous
"""Trainium2 Bass kernel for nn_Classical_autoencoder (patch MLP autoencoder + cosine fold).

Contract: kernel(**inputs) takes FULL inputs (img (32,1,512,512), W1 (16,4), b1 (4,),
W2 (4,4), b2 (4,), W3 (4,16), b3 (16,)) and returns the FULL (32,512,512) output.
Internally: pure data-parallel over 8 NeuronCores, 4 images per core.

v5 design:
  - host precomputes the full im2col tensor X [128, ci, l, t, j] (all matmul /
    elementwise reads contiguous) and inx = 1/(4*max(|x|,1e-8)) per patch in
    fold layout; inx rides in the same DRAM tensor (one DMA per image)
  - contraction weights are per-ci one-hot maps (g -> partition 4g+ci) so the
    accumulated dot / |y|^2 land directly in fold layout: one sim pipeline per
    image, no DRAM bounce / realign DMA
  - row-fold (T[u] = R[u-1]+R[u]) done with 4 small bf16 matmuls on the PE
    (shift/identity weights) instead of partition-shift DMAs
  - deep software pipeline: every PE instruction's producers are >=1 macro-iter
    old; single 5-deep psum ring for z1/z2/z3 frees a bank for the fold
  - const DMAs issued from the scalar HWDGE queue in parallel with X DMAs
"""

import sys

for _p in ("/opt/trn_rl_repo", "/root/.axon_site/_ro/trn_rl_repo"):
    if _p not in sys.path:
        sys.path.append(_p)

from contextlib import ExitStack

import numpy as np

import concourse.bass as bass
import concourse.tile as tile
from concourse import bacc, mybir

F32 = mybir.dt.float32
BF16 = mybir.dt.bfloat16
ALU = mybir.AluOpType
ACT = mybir.ActivationFunctionType

IMG = 512
NJ = 256  # padded patch-column count
XEL = 4 * 4 * 2 * NJ  # X elements per partition (8192)
NSAMP = 4
NCORES = 8
NIT = NSAMP * 4


def build_nc() -> bass.Bass:
    nc = bacc.Bacc()

    # [s, p, ci, l, t, j] im2col (8192) then inx (512)
    x_d = nc.declare_dram_parameter("xim", [NSAMP, 128, XEL + 512], BF16, isOutput=False)[:]
    # bf16 consts: l1w(512) l2w(128) | l3w(512) cwv(512) foldw(512) = 2176
    cb_d = nc.declare_dram_parameter("cbf", [128, 2176], BF16, isOutput=False)[:]
    cf_d = nc.declare_dram_parameter("cf32", [128, 6], F32, isOutput=False)[:]
    out4 = nc.declare_dram_parameter("out4", [NSAMP, IMG, IMG], F32, isOutput=True)[:]

    with ExitStack() as ctx:
        tc = ctx.enter_context(tile.TileContext(nc))
        consts = ctx.enter_context(tc.tile_pool(name="consts", bufs=1))
        rows = ctx.enter_context(tc.tile_pool(name="rows", bufs=3))
        mlp = ctx.enter_context(tc.tile_pool(name="mlp", bufs=3))
        fold = ctx.enter_context(tc.tile_pool(name="fold", bufs=2))
        ps = ctx.enter_context(tc.tile_pool(name="ps", bufs=1, space="PSUM"))

        xt = {}
        inxt = {}

        def dma_img(s, first=False):
            if s >= NSAMP:
                return
            xx = rows.tile([128, XEL + 512], BF16, tag="xx", name="xx")
            if first:  # split so the first matmuls only wait for ci=0's chunk
                nc.sync.dma_start(out=xx[:, 0 : 8 * NJ], in_=x_d[s, :, 0 : 8 * NJ])
                nc.sync.dma_start(out=xx[:, 8 * NJ :], in_=x_d[s, :, 8 * NJ :])
            else:
                nc.sync.dma_start(out=xx, in_=x_d[s, :, :])
            xt[s] = xx[:, 0:XEL].rearrange("p (ci l t j) -> p ci l t j", ci=4, l=4, t=2)
            inxt[s] = xx[:, XEL:].rearrange("p (t j) -> p t j", t=2)

        cb = consts.tile([128, 2176], BF16)
        nc.scalar.dma_start(out=cb[:, 0:640], in_=cb_d[:, 0:640])  # l1w+l2w first
        dma_img(0, first=True)
        cf = consts.tile([128, 6], F32)
        nc.scalar.dma_start(out=cf, in_=cf_d[:, :])
        nc.scalar.dma_start(out=cb[:, 640:], in_=cb_d[:, 640:])
        dma_img(1)  # issued last: image 0's chunks win the HBM bandwidth race
        cbr = cb.rearrange("p (a b) -> p a b", a=17)
        l1w = cbr[:, 0:4]
        l2w = cbr[:, 4]
        l3w = cbr[:, 5:9]
        cwv = cbr[:, 9:13]
        fwD = cbr[:, 13]  # diag(2,1,..,1)
        fwC = cbr[:, 14]  # [p,m]=1 iff p==m-1
        fwA = cbr[:, 15]  # diag(1,..,1,2)
        fwB = cbr[:, 16]  # diag(1,..,1,0)
        b3v = cf[:, 0:4]
        b1v = cf[:, 4:5]
        b2v = cf[:, 5:6]
        epsv = consts.tile([128, 1], F32)
        nc.vector.memset(epsv, 1e-12)

        h1t = {}
        h2t = {}
        pt = {}
        ctdt = {}
        ctyt = {}
        p1t = {}

        def stageA(k):  # z1 + h1
            s, ci = divmod(k, 4)
            z1 = ps.tile([128, 2, NJ], F32, tag="zz", bufs=5, name="z1")
            for l in range(4):
                nc.tensor.matmul(z1, l1w[:, l, :], xt[s][:, ci, l], start=(l == 0), stop=(l == 3))
            h1 = mlp.tile([128, 2, NJ], BF16, tag="h1")
            nc.scalar.activation(h1, z1, ACT.Relu, bias=b1v)
            h1t[k] = h1

        def stageB(k):  # z2 + h2
            z2 = ps.tile([128, 2, NJ], F32, tag="zz", bufs=5, name="z2")
            nc.tensor.matmul(z2, l2w, h1t.pop(k), start=True, stop=True)
            h2 = mlp.tile([128, 2, NJ], BF16, tag="h2")
            nc.scalar.activation(h2, z2, ACT.Relu, bias=b2v)
            h2t[k] = h2

        def stageC(k):  # z3, yv, prod, ysq
            s, ci = divmod(k, 4)
            h2 = h2t.pop(k)
            z3s = []
            for l in range(4):
                z3 = ps.tile([128, 2, NJ], F32, tag="zz", bufs=5, name="z3")
                nc.tensor.matmul(z3, l3w[:, l, :], h2, start=True, stop=True)
                z3s.append(z3)
            Y = mlp.tile([128, 4, 2, NJ], BF16, tag="Y")
            nc.scalar.activation(Y[:, 0], z3s[0], ACT.Relu, bias=b3v[:, 0:1])
            nc.scalar.activation(Y[:, 1], z3s[1], ACT.Relu, bias=b3v[:, 1:2])
            nc.scalar.activation(Y[:, 2], z3s[2], ACT.Relu, bias=b3v[:, 2:3])
            nc.vector.tensor_scalar(Y[:, 3], z3s[3], b3v[:, 3:4], 0.0, ALU.add, ALU.max)
            P = mlp.tile([128, 4, 2, NJ], BF16, tag="P")
            nc.vector.tensor_tensor(P, xt[s][:, ci], Y, ALU.mult)
            YS = mlp.tile([128, 4, 2, NJ], BF16, tag="YS")
            nc.vector.tensor_tensor(YS, Y, Y, ALU.mult)
            pt[k] = (P, YS)

        def stageD(k):  # contractions (issued one iter late; producers all ready)
            s, ci = divmod(k, 4)
            P, YS = pt.pop(k)
            if ci == 0:
                ctdt[s] = ps.tile([128, 2, NJ], F32, tag="ctd", bufs=1, name="ctd")
                ctyt[s] = ps.tile([128, 2, NJ], F32, tag="cty", bufs=1, name="cty")
            ctd, cty = ctdt[s], ctyt[s]
            for l in range(4):
                nc.tensor.matmul(
                    ctd, cwv[:, ci, :], P[:, l],
                    start=(ci == 0 and l == 0), stop=(ci == 3 and l == 3),
                )
            for l in range(4):
                nc.tensor.matmul(
                    cty, cwv[:, ci, :], YS[:, l],
                    start=(ci == 0 and l == 0), stop=(ci == 3 and l == 3),
                )

        def tail1(s):  # prefetch, sim pipeline, col fold
            dma_img(s + 2)
            ctd = ctdt.pop(s)
            cty = ctyt.pop(s)
            u = fold.tile([128, 2, NJ], F32, tag="u")
            nc.vector.tensor_tensor(u, ctd, inxt[s], ALU.mult)
            av = fold.tile([128, 2, NJ], F32, tag="av")
            nc.vector.tensor_scalar(av, cty, epsv[:, :], None, ALU.add)
            q = fold.tile([128, 2, NJ], F32, tag="q")
            nc.vector.reciprocal_approx_fast(q, av)
            sq = fold.tile([128, 2, NJ], F32, tag="sq")
            nc.scalar.activation(sq, q, ACT.Sqrt)  # 1/|y|
            eng = nc.vector if s == NSAMP - 1 else nc.gpsimd  # fast path on tail
            simt = fold.tile([128, 2, NJ], BF16, tag="simt")
            eng.tensor_tensor(simt, u, sq, ALU.mult)

            rf = fold.tile([128, 2, NJ], BF16, tag="rf")
            eng.tensor_tensor(
                rf[:, :, 1:255], simt[:, :, 0:254], simt[:, :, 1:255], ALU.add
            )
            nc.scalar.activation(rf[:, :, 0:1], simt[:, :, 0:1], ACT.Copy, scale=2.0)
            nc.scalar.activation(
                rf[:, :, 255:256], simt[:, :, 254:255], ACT.Copy, scale=2.0
            )
            p1t[s] = rf

        def tail2(s):  # row fold on PE + col-duplicate + store
            rf = p1t.pop(s)
            tf = ps.tile([128, 2, NJ], F32, tag="tfp", bufs=1, name="tf")
            # tf[:,0][m] = rf[m-1,1] + rf[m,0] (m=0: 2*rf[0,0])
            nc.tensor.matmul(tf[:, 0], fwD, rf[:, 0, :], start=True, stop=False)
            nc.tensor.matmul(tf[:, 0], fwC, rf[:, 1, :], start=False, stop=True)
            # tf[:,1][m] = rf[m,0] + rf[m,1] (m=127: 2*rf[127,0])
            nc.tensor.matmul(tf[:, 1], fwA, rf[:, 0, :], start=True, stop=False)
            nc.tensor.matmul(tf[:, 1], fwB, rf[:, 1, :], start=False, stop=True)
            up2 = fold.tile([128, 2, 512], F32, tag="up2")
            up2r = up2.rearrange("p lu (v cv) -> p lu cv v", cv=2)
            nc.vector.tensor_copy(up2r[:, :, 0, :], tf)
            nc.scalar.activation(up2r[:, :, 1, :], tf, ACT.Copy)
            # out rows r = 4p + 2lu + ru: row-duplicate = 2 DMAs reading up2
            for ru in range(2):
                nc.sync.dma_start(
                    out=bass.AP(
                        tensor=out4.tensor,
                        offset=out4.offset + s * IMG * IMG + ru * IMG,
                        ap=[[4 * IMG, 128], [2 * IMG, 2], [1, IMG]],
                    ),
                    in_=bass.AP(
                        tensor=up2.tensor,
                        offset=up2.offset,
                        ap=[[1024, 128], [512, 2], [1, 512]],
                    ),
                )

        # ---- deep software pipeline ----
        stageA(0)
        stageA(1)
        stageB(0)
        for k in range(NIT):
            stageC(k)
            if k + 1 < NIT:
                stageB(k + 1)
            if k + 2 < NIT:
                stageA(k + 2)
            if k - 1 >= 0:
                stageD(k - 1)
                s1, ci1 = divmod(k - 1, 4)
                if ci1 == 3:
                    tail1(s1)
            if k % 4 == 2 and k >= 6:
                tail2(k // 4 - 1)
        stageD(NIT - 1)
        tail1(NSAMP - 1)
        tail2(NSAMP - 1)

    nc.finalize()
    return nc


def make_weight_inputs(W1, b1, W2, b2, W3, b3):
    W1 = np.asarray(W1, np.float32)
    W2 = np.asarray(W2, np.float32)
    W3 = np.asarray(W3, np.float32)
    b1 = np.asarray(b1, np.float32)
    b2 = np.asarray(b2, np.float32)
    b3 = np.asarray(b3, np.float32)
    l1w = np.zeros((128, 4, 128), np.float32)
    l2w = np.zeros((128, 128), np.float32)
    l3w = np.zeros((128, 4, 128), np.float32)
    b3v = np.zeros((128, 4), np.float32)
    cwv = np.zeros((128, 4, 128), np.float32)
    for g in range(32):
        for k in range(4):
            for l in range(4):
                for c in range(4):
                    l1w[32 * k + g, l, 32 * c + g] = W1[4 * k + l, c]
                    l3w[32 * c + g, l, 32 * k + g] = W3[c, 4 * k + l]
                b3v[32 * k + g, l] = b3[4 * k + l]
            for ci in range(4):
                cwv[32 * k + g, ci, 4 * g + ci] = 1.0
        for c in range(4):
            for c2 in range(4):
                l2w[32 * c + g, 32 * c2 + g] = W2[c, c2]
    b1v = np.repeat(b1, 32).reshape(128, 1).astype(np.float32)
    b2v = np.repeat(b2, 32).reshape(128, 1).astype(np.float32)
    # fold-row weights
    fwD = np.diag(np.r_[2.0, np.ones(127)]).astype(np.float32)
    fwC = np.zeros((128, 128), np.float32)
    fwC[np.arange(127), np.arange(1, 128)] = 1.0  # [p, m]=1 iff p==m-1
    fwA = np.diag(np.r_[np.ones(127), 2.0]).astype(np.float32)
    fwB = np.diag(np.r_[np.ones(127), 0.0]).astype(np.float32)
    bf = _bf16()
    cbf = np.concatenate(
        [
            l1w.reshape(128, 512), l2w, l3w.reshape(128, 512), cwv.reshape(128, 512),
            fwD, fwC, fwA, fwB,
        ],
        axis=1,
    ).astype(bf)
    cf32 = np.concatenate([b3v, b1v, b2v], axis=1).astype(np.float32)
    return {"cbf": cbf, "cf32": cf32}


_NC = None


def get_nc():
    global _NC
    if _NC is None:
        _NC = build_nc()
    return _NC


def _bf16():
    import ml_dtypes

    return ml_dtypes.bfloat16


def gather_inputs(img_n):
    """(n,512,512) f32 -> xim (n,128,8704) bf16: im2col X then inx."""
    n = img_n.shape[0]
    pad = np.zeros((n, IMG + 4, IMG + 2), np.float32)
    pad[:, :IMG, :IMG] = img_n
    p = np.arange(128)
    li = np.arange(8)
    rows_idx = 16 * (p[:, None] % 32) + (p[:, None] // 32) + 2 * li[None, :]
    rws = pad[:, rows_idx, :]  # (n,128,8,514); li = 2ci+t
    rwr = rws.reshape(n, 128, 4, 2, IMG + 2)  # (ci, t, c)
    cols = 2 * np.arange(NJ)[None, :] + np.arange(4)[:, None]  # (l, j) -> 2j+l
    X = rwr[:, :, :, :, cols]  # (n,128,ci,t,l,j)
    X = X.transpose(0, 1, 2, 4, 3, 5)  # (n,128,ci,l,t,j)
    X = np.ascontiguousarray(X).reshape(n, 128, XEL)

    sq = img_n.astype(np.float64) ** 2
    p2 = sq[:, :, 0::2] + sq[:, :, 1::2]
    s4 = p2[:, :, 0:255] + p2[:, :, 1:256]
    r2 = s4[:, 0::2, :] + s4[:, 1::2, :]
    r4 = r2[:, 0:255, :] + r2[:, 1:256, :]  # (n,255,255) = |x|^2
    inx = np.zeros((n, 256, 256), np.float64)
    inx[:, :255, :255] = 1.0 / (4.0 * np.maximum(np.sqrt(r4), 1e-8))
    inx = inx.reshape(n, 128, 512)
    xim = np.concatenate([X, inx], axis=2).astype(_bf16())
    return xim


def build_in_maps(img, W1, b1, W2, b2, W3, b3):
    img = np.asarray(img, np.float32).reshape(32, IMG, IMG)
    wts = make_weight_inputs(W1, b1, W2, b2, W3, b3)
    in_maps = []
    for c in range(NCORES):
        m = {"xim": gather_inputs(img[c * NSAMP : (c + 1) * NSAMP])}
        m.update(wts)
        in_maps.append(m)
    return in_maps


def kernel(img, W1, b1, W2, b2, W3, b3):
    from concourse.bass_utils import run_bass_kernel_spmd

    nc = get_nc()
    in_maps = build_in_maps(img, W1, b1, W2, b2, W3, b3)
    res = run_bass_kernel_spmd(nc, in_maps, list(range(NCORES)))
    out = np.concatenate([res.results[i]["out4"] for i in range(NCORES)], axis=0)
    return out.astype(np.float32)
